# revision 1
# baseline (speedup 1.0000x reference)
"""Batch-parallel dot-product attention for TRN2 (8 NeuronCores).

reference: context[b] = softmax(Q[b] @ K[b].T / sqrt(64)) @ V[b]
with Q,K,V: [32, 2048, 64] fp32.

Sharding: pure data parallel — 4 batches per core, no collectives.

Per-core kernel (per batch, per 1024-query half):
  scores_T[k, q] = (K @ Q^T)/8      computed as lhsT=K^T-slice, rhs=Q^T-slice
  P_T = exp(scores_T)               ScalarE, scale=1/8 fused, bf16 out
  ctx_T[d, q]   = sum_k Vaug^T P_T  PSUM accumulation, Vaug = [V | 1]
  (row 64 of ctx_T = softmax denominator via the ones column)
  transpose ctx_T -> [q, d] via TensorE transpose, divide by denom, DMA out.

Host side pre-transposes Q/K to [d, s] layout, pre-casts to bf16 and
appends the ones column to V so the device does zero layout work.
"""

import numpy as np

import concourse.bass as bass
import concourse.bacc as bacc
import concourse.tile as tile
from concourse import mybir
from concourse.bass_utils import run_bass_kernel_spmd

NCORES = 8
BPC = 4  # batches per core
S = 2048
D = 64
DA = D + 1  # V augmented with ones column
NKT = S // 128  # 16 key tiles of 128
NH = 2  # query halves
HQ = S // NH  # 1024 queries per half
NQC = HQ // 512  # 512-wide matmul chunks per half

FP16 = mybir.dt.float16
F32 = mybir.dt.float32
I16 = mybir.dt.int16

# fp16 exp2 bit-trick constants (DVE path)
EXP_C = 0.6931471805599453 / 8 * 1.4426950408889634  # = log2(e)/8... see below
LOG2E_8 = 1.4426950408889634 / 8.0  # y = score * log2(e)/8 so exp(score/8)=2^y
MAGIC = 1536.0  # 1.5 * 2^10: fp16 round-to-int magic
TCLAMP = 1522.0  # keeps n >= -14 (fp16 normal range)
SBIAS = 15360.0 - MAGIC * 1024.0  # maps t -> fp16 bits of 2^n
PC1, PC2, PC3 = 0.693121033991547, 0.24223731726222145, 0.0559220356472602
DVE_STEPS = ()  # DVE exp offload measured 2.2x WORSE on HW - disabled

_cache = {}


def _build(reps=1):
    if reps in _cache:
        return _cache[reps]

    nc = bacc.Bacc(
        "TRN2",
        target_bir_lowering=False,
        debug=False,
        num_devices=1,
        enable_partition_id=False,
    )

    qt_d = nc.dram_tensor("qt", [BPC, 128, S], FP16, kind="ExternalInput").ap()
    kt_d = nc.dram_tensor("kt", [BPC, 128, S // 2], FP16, kind="ExternalInput").ap()
    # host pre-tiles V-augmented to [BPC, 128, NKT, DA] so the DMA is contiguous
    va_d = nc.dram_tensor("va", [BPC, 128, NKT, DA], FP16, kind="ExternalInput").ap()
    id_d = nc.dram_tensor("ident", [DA, DA], F32, kind="ExternalInput").ap()
    # device writes [BPC, NH, 128, 8*D] contiguously; host re-tiles to [B, S, D]
    out_d = nc.dram_tensor("out", [BPC, NH, 128, 8 * D], F32, kind="ExternalOutput").ap()
    va_v = va_d
    out_v = out_d

    with tile.TileContext(nc) as tc:
        with (
            tc.tile_pool(name="io", bufs=2) as io,
            tc.tile_pool(name="const", bufs=1) as const,
            tc.tile_pool(name="pt", bufs=6) as ptp,
            tc.tile_pool(name="csb", bufs=2) as csbp,
            tc.tile_pool(name="outsb", bufs=2) as outp,
            tc.tile_pool(name="small", bufs=4) as small,
            tc.tile_pool(name="dvet", bufs=2) as dvet,
            tc.tile_pool(name="scps", bufs=2, space="PSUM") as scps,
            tc.tile_pool(name="cxps", bufs=1, space="PSUM") as cxps,
            tc.tile_pool(name="ctps", bufs=2, space="PSUM") as ctps,
        ):
            ident = const.tile([DA, DA], F32)
            nc.sync.dma_start(out=ident, in_=id_d)

            def body():
                pending = []  # deferred drain steps, one emitted per k-step

                def drain(cx, b, h):
                    # split the drain into small closures so the PE/DVE work
                    # interleaves with later k-steps instead of stalling ACT
                    state = {}

                    def start():
                        state["csb"] = csbp.tile([DA, HQ], F32, name="csb")
                        nc.vector.tensor_copy(state["csb"], cx)
                        state["out_sb"] = outp.tile([128, 8 * D], F32, name="out_sb")

                    def chunk(c):
                        def emit():
                            csb, out_sb = state["csb"], state["out_sb"]
                            ct = ctps.tile([128, DA], F32)
                            nc.tensor.transpose(
                                ct, csb[:, c * 128 : (c + 1) * 128], ident
                            )
                            recip = small.tile([128, 1], F32)
                            nc.vector.reciprocal(recip, ct[:, D : D + 1])
                            nc.vector.tensor_scalar_mul(
                                out_sb[:, c * D : (c + 1) * D], ct[:, 0:D], recip
                            )

                        return emit

                    def store():
                        nc.sync.dma_start(out=out_v[b, h], in_=state["out_sb"])

                    return [start] + [chunk(c) for c in range(8)] + [store]

                av_due = []  # (due_step, closure); DVE-exp AVs get extra delay
                step_no = [0]

                def flush_av(final=False):
                    rest = []
                    for due, fn in av_due:
                        if final or due <= step_no[0]:
                            fn()
                        else:
                            rest.append((due, fn))
                    av_due[:] = rest

                def dve_exp(sc, pt):
                    """pt = exp(sc/8) on the Vector engine via fp16 2^y bits.

                    Ops chosen for DVE perf modes: all-fp16 tensor_scalar runs
                    4x, all-fp16 tensor_tensor 2x; scalar_tensor_tensor and
                    mixed-dtype ops would fall to 1x.
                    """
                    y16 = dvet.tile([128, HQ], FP16, name="y16")
                    nc.vector.tensor_scalar_mul(y16, sc, LOG2E_8)
                    t16 = dvet.tile([128, HQ], FP16, name="t16")
                    nc.vector.tensor_scalar(
                        t16, y16, MAGIC, TCLAMP,
                        op0=mybir.AluOpType.add, op1=mybir.AluOpType.max,
                    )
                    n16 = dvet.tile([128, HQ], FP16, name="n16")
                    nc.vector.tensor_scalar_sub(n16, t16, MAGIC)
                    s16 = dvet.tile([128, HQ], I16, name="s16")
                    nc.vector.tensor_scalar(
                        s16, t16, 1024.0, SBIAS,
                        op0=mybir.AluOpType.mult, op1=mybir.AluOpType.add,
                    )
                    f16 = dvet.tile([128, HQ], FP16, name="f16")
                    nc.vector.tensor_tensor(
                        f16, y16, n16, op=mybir.AluOpType.subtract
                    )
                    a16 = dvet.tile([128, HQ], FP16, name="a16")
                    nc.vector.tensor_scalar(
                        a16, f16, PC3, PC2,
                        op0=mybir.AluOpType.mult, op1=mybir.AluOpType.add,
                    )
                    nc.vector.tensor_tensor(a16, a16, f16, op=mybir.AluOpType.mult)
                    nc.vector.tensor_scalar_add(a16, a16, PC1)
                    nc.vector.tensor_tensor(a16, a16, f16, op=mybir.AluOpType.mult)
                    nc.vector.tensor_scalar_add(a16, a16, 1.0)
                    nc.vector.tensor_tensor(
                        pt, a16, s16.bitcast(FP16), op=mybir.AluOpType.mult
                    )

                for b in range(BPC):
                    qt_sb = io.tile([128, S], FP16)
                    nc.sync.dma_start(out=qt_sb, in_=qt_d[b])
                    kt_sb = io.tile([128, S // 2], FP16)
                    nc.sync.dma_start(out=kt_sb, in_=kt_d[b])
                    va_sb = io.tile([128, NKT, DA], FP16)
                    nc.sync.dma_start(out=va_sb, in_=va_v[b])

                    for h in range(NH):
                        cx = cxps.tile([DA, HQ], F32)
                        for step, (t, qc) in enumerate(
                            [(t, qc) for t in range(NKT // 2) for qc in range(NQC)]
                        ):
                            k = step  # step index for DVE_STEPS / deferral
                            sc = scps.tile([128, HQ], F32)
                            q0 = h * HQ + qc * 512
                            nc.tensor.matmul(
                                sc[:, 0:512],
                                lhsT=kt_sb[0:64, t * 128 : (t + 1) * 128],
                                rhs=qt_sb[0:64, q0 : q0 + 512],
                                start=True,
                                stop=True,
                            )
                            nc.tensor.matmul(
                                sc[:, 512:1024],
                                lhsT=kt_sb[64:128, t * 128 : (t + 1) * 128],
                                rhs=qt_sb[64:128, q0 : q0 + 512],
                                start=True,
                                stop=True,
                                tile_position=(64, 0),
                            )
                            # previous k-steps' AV matmuls go after this
                            # k-step's score matmuls so PE never waits on
                            # the exp that feeds them.
                            step_no[0] += 1
                            flush_av()
                            if pending:
                                pending.pop(0)()
                            pt = ptp.tile([128, HQ], FP16)
                            if k in DVE_STEPS:
                                dve_exp(sc, pt)
                            else:
                                nc.scalar.activation(
                                    out=pt,
                                    in_=sc,
                                    func=mybir.ActivationFunctionType.Exp,
                                    scale=0.125,
                                )

                            def av(cx=cx, pt=pt, t=t, qc=qc):
                                for j in range(2):
                                    nc.tensor.matmul(
                                        cx[:, qc * 512 : (qc + 1) * 512],
                                        lhsT=va_sb[:, 2 * t + j, :],
                                        rhs=pt[:, j * 512 : (j + 1) * 512],
                                        start=(t == 0 and j == 0),
                                        stop=(t == NKT // 2 - 1 and j == 1),
                                        skip_group_check=True,
                                    )

                            av_due.append(
                                (step_no[0] + (4 if k in DVE_STEPS else 1), av)
                            )
                        flush_av(final=True)
                        pending.extend(drain(cx, b, h))
                for p in pending:
                    p()

            if reps == 1:
                body()
            else:
                with tc.For_i(
                    0,
                    reps,
                    1,
                    hint_engines=(
                        mybir.EngineType.PE,
                        mybir.EngineType.Activation,
                        mybir.EngineType.DVE,
                        mybir.EngineType.SP,
                    ),
                ):
                    body()

    nc.compile()
    _cache[reps] = nc
    return nc


def _prep_core_inputs(query, key, value, core):
    sl = slice(core * BPC, (core + 1) * BPC)
    # cast-on-gather (single pass), then pack in fp16 (half the host traffic)
    qT = query[sl].transpose(0, 2, 1).astype(np.float16)  # [BPC, D, S]
    q = np.concatenate([qT, qT], axis=1)  # duplicate across both partition halves
    kk = key[sl].transpose(0, 2, 1).astype(np.float16).reshape(BPC, D, NKT, 128)
    k = np.ascontiguousarray(
        np.concatenate([kk[:, :, 0::2], kk[:, :, 1::2]], axis=1)
    ).reshape(BPC, 128, S // 2)  # rows 0-63: even k-tiles, 64-127: odd
    v16 = value[sl].astype(np.float16)
    ones = np.ones((BPC, S, 1), dtype=np.float16)
    va = np.concatenate([v16, ones], axis=2)
    # [BPC, S, DA] -> [BPC, 128, NKT, DA]: row s = n*128 + p lives at [p, n]
    va_t = np.ascontiguousarray(va.reshape(BPC, NKT, 128, DA).transpose(0, 2, 1, 3))
    return {
        "qt": q,
        "kt": k,
        "va": va_t,
        "ident": np.eye(DA, dtype=np.float32),
    }


def run(query, key, value, trace=False):
    nc = _build()
    query = np.asarray(query, dtype=np.float32)
    key = np.asarray(key, dtype=np.float32)
    value = np.asarray(value, dtype=np.float32)
    in_maps = [_prep_core_inputs(query, key, value, c) for c in range(NCORES)]
    res = run_bass_kernel_spmd(nc, in_maps, core_ids=list(range(NCORES)))
    outs = []
    for c in range(NCORES):
        o = np.asarray(res.results[c]["out"])  # [BPC, NH, 128, 8*D]
        o = o.reshape(BPC, NH, 128, 8, D).transpose(0, 1, 3, 2, 4).reshape(BPC, S, D)
        outs.append(o)
    return np.concatenate(outs, axis=0).astype(np.float32), res


def kernel(query, key, value):
    out, _ = run(query, key, value)
    return out

